# revision 7
# baseline (speedup 1.0000x reference)
"""Trainium2 Bass kernel: causal attention with 3D (Rodrigues) RoPE.

Sharding: tensor-parallel over heads (2 heads/core on 8 cores) for
QKV projection + RoPE + SDPA, then an AllToAll redistributes attention
outputs so the output projection is sharded over tokens (512/core).

Layouts (per core, all matmuls in float32r):
  x^T       [1536, 4096]   tokens on the free axis
  q^T,k^T   [96, 4096]     per head, head-dim on partitions (plane-major
                           triplet order so RoPE shifts are 32-row blocks)
  V         [4096, 194]    token tiles on partitions; cols = v_h0|1|v_h1|1
                           (the 1-columns give the softmax denominator as
                           row 96 of the PV matmul output)
  S^T       [tk=128, tq=512] so softmax's reduction axis is the PE
                           contraction axis -> no transposes anywhere.
"""

import sys

sys.path.insert(0, "/opt/trn_rl_repo")

import numpy as np

D_MODEL, N_HEADS, HEAD_DIM, MAX_POS = 1536, 16, 96, 4096
B, T = 2, 2048
NTOK = B * T                      # 4096
NCORES = 8
HPC = N_HEADS // NCORES           # 2 heads per core
NTRIP = HEAD_DIM // 3             # 32 triplets
KT = D_MODEL // 128               # 12 contraction tiles
NCH = NTOK // 512                 # 8 token chunks of 512
TQC = T // 512                    # 4 query chunks per batch
SCALE = 1.0 / np.sqrt(HEAD_DIM)

_CACHE = {}


def _build_nc():
    import concourse.bass as bass
    import concourse.mybir as mybir
    import concourse.tile as tile
    from concourse import bacc

    f32 = mybir.dt.float32
    f32r = mybir.dt.float32r

    nc = bacc.Bacc("TRN2", target_bir_lowering=False, debug=False,
                   enable_asserts=False, num_devices=NCORES)

    xT = nc.dram_tensor("xT", [D_MODEL, NTOK], f32r, kind="ExternalInput").ap()
    wqkT = nc.dram_tensor("wqkT", [D_MODEL, 4 * 96], f32r, kind="ExternalInput").ap()
    wvT = nc.dram_tensor("wvT", [D_MODEL, 256], f32r, kind="ExternalInput").ap()
    woT = nc.dram_tensor("woT", [D_MODEL, D_MODEL], f32r, kind="ExternalInput").ap()
    cco = nc.dram_tensor("cco", [96, 3, T], f32, kind="ExternalInput").ap()
    msk = nc.dram_tensor("msk", [128, 4, 512], f32r, kind="ExternalInput").ap()
    out = nc.dram_tensor("out", [D_MODEL, 512], f32, kind="ExternalOutput").ap()

    with tile.TileContext(nc) as tc:
        with tc.tile_pool(name="dram", bufs=1, space="DRAM") as dram:
            a2a_in = dram.tile([NCH, HPC * 96, 512], f32r, name="a2a_in")
            a2a_out = dram.tile([NCH, HPC * 96, 512], f32r, name="a2a_out")

            with tc.tile_pool(name="ph12", bufs=1) as pp:
                qk_rot = [pp.tile([96, NTOK], f32r, tag=f"qkrot{i}",
                                  name=f"qkrot{i}") for i in range(4)]
                v_sb = pp.tile([128, NTOK // 128, 194], f32r, tag="vsb")

                # ------------ phase 1: qkv projection + rope ------------
                with tc.tile_pool(name="ph1", bufs=1) as p1, \
                     tc.tile_pool(name="ph1s", bufs=2) as p1s, \
                     tc.tile_pool(name="ps_qk", bufs=6, space="PSUM") as ps_qk, \
                     tc.tile_pool(name="ps_v", bufs=2, space="PSUM") as ps_v:
                    wqk_sb = p1.tile([128, KT, 4 * 96], f32r, tag="wqk")
                    wv_sb = p1.tile([128, KT, 256], f32r, tag="wv")
                    nc.sync.dma_start(
                        wqk_sb[:], wqkT.rearrange("(k p) c -> p k c", p=128))
                    nc.sync.dma_start(
                        wv_sb[:], wvT.rearrange("(k p) c -> p k c", p=128))

                    for ch in range(NCH):
                        coff = (ch % TQC) * 512   # position within batch
                        c_sl = p1s.tile([96, 3, 512], f32, tag="csl")
                        nc.sync.dma_start(c_sl[:], cco[:, :, coff:coff + 512])
                        xt = []
                        for kt in range(KT):
                            t = p1s.tile([128, 512], f32r, tag=f"xt{kt}",
                                         name=f"xt{kt}")
                            nc.sync.dma_start(
                                t[:], xT[kt * 128:(kt + 1) * 128,
                                         ch * 512:(ch + 1) * 512])
                            xt.append(t)
                        # q/k projection: 4 M-tiles of 96 (qh0, qh1, kh0, kh1)
                        for m in range(4):
                            ps = ps_qk.tile([96, 512], f32, tag="qk")
                            for kt in range(KT):
                                nc.tensor.matmul(
                                    ps[:], wqk_sb[:, kt, m * 96:(m + 1) * 96],
                                    xt[kt][:], start=(kt == 0),
                                    stop=(kt == KT - 1))
                            raw = p1s.tile([96, 512], f32, tag="raw")
                            nc.vector.tensor_copy(raw[:], ps[:])
                            # rope: dst = C0*raw + C1*rot1(raw) + C2*rot2(raw)
                            dst = qk_rot[m][:, ch * 512:(ch + 1) * 512]
                            nc.vector.tensor_tensor(
                                dst, raw[:], c_sl[:, 0, :],
                                mybir.AluOpType.mult)
                            g1 = p1s.tile([96, 512], f32, tag="g1")
                            nc.sync.dma_start(g1[0:64, :], raw[32:96, :])
                            nc.sync.dma_start(g1[64:96, :], raw[0:32, :])
                            g2 = p1s.tile([96, 512], f32, tag="g2")
                            nc.sync.dma_start(g2[0:32, :], raw[64:96, :])
                            nc.sync.dma_start(g2[32:96, :], raw[0:64, :])
                            nc.vector.tensor_tensor(
                                g1[:], g1[:], c_sl[:, 1, :],
                                mybir.AluOpType.mult)
                            nc.vector.tensor_tensor(
                                dst, dst, g1[:], mybir.AluOpType.add)
                            nc.vector.tensor_tensor(
                                g2[:], g2[:], c_sl[:, 2, :],
                                mybir.AluOpType.mult)
                            nc.vector.tensor_tensor(
                                dst, dst, g2[:], mybir.AluOpType.add)
                        # v projection: 4 token tiles of 128, N=256 (padded)
                        for ts_ in range(4):
                            psv = ps_v.tile([128, 256], f32, tag="v")
                            for kt in range(KT):
                                nc.tensor.matmul(
                                    psv[:],
                                    xt[kt][:, ts_ * 128:(ts_ + 1) * 128],
                                    wv_sb[:, kt, :], start=(kt == 0),
                                    stop=(kt == KT - 1))
                            g = ch * 4 + ts_
                            nc.scalar.activation(
                                v_sb[:, g, :], psv[:, 0:194],
                                mybir.ActivationFunctionType.Copy)
                            nc.vector.memset(v_sb[:, g, 96:97].bitcast(f32), 1.0)
                            nc.vector.memset(v_sb[:, g, 193:194].bitcast(f32), 1.0)

                # ------------ phase 2: attention ------------
                with tc.tile_pool(name="ph2", bufs=3) as p2, \
                     tc.tile_pool(name="ph2b", bufs=2) as p2b, \
                     tc.tile_pool(name="ps_s", bufs=4, space="PSUM") as ps_s, \
                     tc.tile_pool(name="ps_pv", bufs=2, space="PSUM") as ps_pv:
                    m_sb = p2.tile([128, 4, 512], f32r, tag="msb", bufs=1)
                    nc.sync.dma_start(m_sb[:], msk[:])

                    for h in range(HPC):
                        for b in range(B):
                            for cl in range(TQC):
                                qoff = b * T + cl * 512
                                pv = ps_pv.tile([128, 512], f32, tag="pv")
                                ntk = 4 * cl + 4
                                for tt in range(ntk):
                                    koff = b * T + tt * 128
                                    sp = ps_s.tile([128, 512], f32, tag="s")
                                    nc.tensor.matmul(
                                        sp[:],
                                        qk_rot[2 + h][:, koff:koff + 128],
                                        qk_rot[h][:, qoff:qoff + 512],
                                        start=True, stop=True)
                                    pt = p2.tile([128, 512], f32r, tag="p")
                                    nc.scalar.activation(
                                        pt[:], sp[:],
                                        mybir.ActivationFunctionType.Exp)
                                    if tt >= 4 * cl:
                                        nc.vector.tensor_tensor(
                                            pt[:], pt[:],
                                            m_sb[:, tt - 4 * cl, :],
                                            mybir.AluOpType.mult)
                                    nc.tensor.matmul(
                                        pv[0:97, :],
                                        v_sb[:, b * 16 + tt,
                                             h * 97:h * 97 + 97],
                                        pt[:], start=(tt == 0),
                                        stop=(tt == ntk - 1))
                                linv = p2b.tile([1, 512], f32, tag="linv")
                                nc.vector.reciprocal(linv[:], pv[96:97, :])
                                brow = p2b.tile([96, 512], f32, tag="brow")
                                nc.gpsimd.partition_broadcast(brow[:], linv[:])
                                att = p2b.tile([96, 512], f32r, tag="att")
                                nc.vector.tensor_tensor(
                                    att[:], pv[0:96, :], brow[:],
                                    mybir.AluOpType.mult)
                                nc.sync.dma_start(
                                    a2a_in[b * TQC + cl,
                                           h * 96:(h + 1) * 96, :],
                                    att[:])

            # ------------ phase 3: A2A + output projection ------------
            nc.gpsimd.collective_compute(
                "AllToAll", mybir.AluOpType.bypass,
                replica_groups=[list(range(NCORES))],
                ins=[a2a_in.opt()], outs=[a2a_out.opt()])

            a2a_flat = a2a_out[:].rearrange("a b c -> (a b) c")
            with tc.tile_pool(name="ph3", bufs=1) as p3, \
                 tc.tile_pool(name="ps_o", bufs=7, space="PSUM") as ps_o:
                att2 = []
                for et in range(KT):
                    t = p3.tile([128, 512], f32r, tag=f"att2_{et}",
                                name=f"att2_{et}")
                    nc.sync.dma_start(t[:],
                                      a2a_flat[et * 128:(et + 1) * 128, :])
                    att2.append(t)
                for grp in range(2):
                    pos = [ps_o.tile([128, 512], f32, tag="o",
                                     name=f"po_{grp}_{d6}") for d6 in range(6)]
                    for et in range(KT):
                        wot = p3.tile([128, 768], f32r, tag="wos", bufs=4)
                        nc.sync.dma_start(
                            wot[:], woT[et * 128:(et + 1) * 128,
                                        grp * 768:(grp + 1) * 768])
                        for d6 in range(6):
                            nc.tensor.matmul(
                                pos[d6][:], wot[:, d6 * 128:(d6 + 1) * 128],
                                att2[et][:], start=(et == 0),
                                stop=(et == KT - 1), skip_group_check=True)
                    for d6 in range(6):
                        dt_ = grp * 6 + d6
                        ot = p3.tile([128, 512], f32, tag="ot", bufs=3)
                        nc.vector.tensor_copy(ot[:], pos[d6][:])
                        nc.sync.dma_start(
                            out[dt_ * 128:(dt_ + 1) * 128, :], ot[:])

    nc.compile()
    return nc


def _plane_major(w):
    """Reorder head-dim rows 3k+i -> 32i+k (per 96-row head block)."""
    idx = np.empty(96, dtype=np.int64)
    for i in range(3):
        for k in range(NTRIP):
            idx[32 * i + k] = 3 * k + i
    return w[idx]


def _prep_inputs(x, w_qkv, w_o, Rs):
    x = np.asarray(x, dtype=np.float32)
    w_qkv = np.asarray(w_qkv, dtype=np.float32)
    w_o = np.asarray(w_o, dtype=np.float32)
    Rs = np.asarray(Rs, dtype=np.float32)

    xT = np.ascontiguousarray(x.reshape(NTOK, D_MODEL).T)

    # rope coefficients, plane-major rows: C[d, delta, t]
    R = Rs[:T]                                   # (T, 32, 3, 3)
    cco = np.empty((96, 3, T), dtype=np.float32)
    for d in range(3):
        for i in range(3):
            cco[32 * i:32 * i + 32, d, :] = R[:, :, i, (i + d) % 3].T

    # causal masks for the 4 diagonal sub-tiles
    msk = np.empty((128, 4, 512), dtype=np.float32)
    j = np.arange(128)[:, None]
    i = np.arange(512)[None, :]
    for m in range(4):
        msk[:, m, :] = (m * 128 + j <= i).astype(np.float32)

    woT = np.ascontiguousarray(w_o.T)

    def w_row(s, h):                             # rows of w_qkv for (q/k/v, head)
        base = (s * N_HEADS + h) * HEAD_DIM
        return w_qkv[base:base + HEAD_DIM]

    in_maps = []
    for c in range(NCORES):
        h0, h1 = 2 * c, 2 * c + 1
        wqk = np.concatenate([
            _plane_major(w_row(0, h0)) * SCALE,
            _plane_major(w_row(0, h1)) * SCALE,
            _plane_major(w_row(1, h0)),
            _plane_major(w_row(1, h1)),
        ], axis=0)                               # [384, 1536]
        wqkT = np.ascontiguousarray(wqk.T)       # [1536, 384]
        wv = np.zeros((256, D_MODEL), dtype=np.float32)
        wv[0:96] = w_row(2, h0)
        wv[97:193] = w_row(2, h1)
        wvT = np.ascontiguousarray(wv.T)         # [1536, 256]
        in_maps.append({
            "xT": xT, "wqkT": wqkT, "wvT": wvT, "woT": woT,
            "cco": cco, "msk": msk,
        })
    return in_maps


def kernel(x, w_qkv, w_o, Rs):
    from concourse import bass_utils

    if "nc" not in _CACHE:
        _CACHE["nc"] = _build_nc()
    nc = _CACHE["nc"]
    in_maps = _prep_inputs(x, w_qkv, w_o, Rs)
    res = bass_utils.run_bass_kernel_spmd(
        nc, in_maps, core_ids=list(range(NCORES)))
    full_T = np.concatenate([res.results[c]["out"] for c in range(NCORES)],
                            axis=1)              # [1536, 4096]
    return np.ascontiguousarray(full_T.T).reshape(B, T, D_MODEL)
